# revision 2
# baseline (speedup 1.0000x reference)
"""Chebyshev GCN kernel for Trainium2 (8 NeuronCores, data-parallel over batch).

Math (per batch b):
  masked[k] = cheb[k] * STDG[b]            (elementwise, [N,N])
  Y[k]      = masked[k].T @ x[b]           ([N, C*T] contraction over nodes m)
  out[b]    = relu( sum_k  Y[k] contracted with theta[k] over C )

Mapping per core (2 batches/core):
  stage 1 (PE, f32r): lhsT = x tiles [m_part=128, (c,t2)=128], rhs = masked
           tiles [m_part=128, n=512]  ->  psum Y [(c,t2)=128, n=512],
           accumulated over 8 m-tiles.  t2 = t-pair (t0, t0+12) so the
           stationary free dim packs c(64) x t(2) = 128 partitions out.
  stage 2 (PE, bf16): lhsT = Y[k] [(c,t2), n_chunk=128], rhs = block-diag
           theta [(c,t2)=128, (o,t2)=128] -> psum out [n=128, (o,t2)=128],
           accumulated over k.  ReLU fused into the PSUM->SBUF copy that
           scatters (o,t2) columns into a contiguous [n, (o,t)] tile.
"""

import numpy as np
import ml_dtypes

import concourse.bass as bass
import concourse.mybir as mybir
from concourse import bacc
from concourse.tile import TileContext
from concourse.bass_utils import run_bass_kernel_spmd

B, N, C, T, K, O = 16, 1024, 64, 24, 3, 64
NCORES = 8
BL = B // NCORES  # batches per core
MT = N // 128  # m tiles
TP = T // 2  # t-pairs
NH = 2  # n halves
NHW = N // NH  # 512
F32 = mybir.dt.float32
F32R = mybir.dt.float32r
BF16 = mybir.dt.bfloat16

_cache = {}


def build_nc():
    nc = bacc.Bacc("TRN2", target_bir_lowering=False, debug=False, num_devices=NCORES)
    x_d = nc.dram_tensor("x", [BL, N, C, T], F32, kind="ExternalInput")
    sg_d = nc.dram_tensor("stdg", [BL, N, N], F32, kind="ExternalInput")
    ch_d = nc.dram_tensor("cheb", [K, N, N], F32, kind="ExternalInput")
    tb_d = nc.dram_tensor("thetab", [K, 128, 128], BF16, kind="ExternalInput")
    y_d = nc.dram_tensor("y", [BL, N, C, T], F32, kind="ExternalOutput")

    with TileContext(nc) as tc:
        with (
            tc.tile_pool(name="xp", bufs=MT) as xp,
            tc.tile_pool(name="xstg", bufs=2) as xstg,
            tc.tile_pool(name="cstg", bufs=3) as cstg,
            tc.tile_pool(name="sstg", bufs=3) as sstg,
            tc.tile_pool(name="mk", bufs=K * MT + 3) as mkp,
            tc.tile_pool(name="yp", bufs=2 * K) as yp,
            tc.tile_pool(name="tbp", bufs=1) as tbp,
            tc.tile_pool(name="asmp", bufs=6) as asmp,
            tc.tile_pool(name="psy", bufs=2 * K, space="PSUM") as psy,
            tc.tile_pool(name="pso", bufs=2, space="PSUM") as pso,
        ):
            tb = tbp.tile([128, K * 128], BF16)
            for k in range(K):
                nc.sync.dma_start(out=tb[:, k * 128 : (k + 1) * 128], in_=tb_d[k])

            for b in range(BL):
                # ---- load x[b], round to f32r ----
                xts = []
                for m in range(MT):
                    xs = xstg.tile([128, C * T], F32, name=f"xs_{b}_{m}", tag="xs")
                    nc.sync.dma_start(
                        out=xs[:],
                        in_=x_d[b, m * 128 : (m + 1) * 128].rearrange(
                            "p c t -> p (c t)"
                        ),
                    )
                    xt = xp.tile([128, C * T], F32R, name=f"xt_{b}_{m}", tag="xt")
                    nc.vector.tensor_copy(out=xt[:], in_=xs[:])
                    xts.append(xt)

                for nh in range(NH):
                    nsl = slice(nh * NHW, (nh + 1) * NHW)
                    # ---- phase A: masked tiles ----
                    mks = {}
                    for m in range(MT):
                        ss = sstg.tile([128, NHW], F32, name=f"ss_{b}_{nh}_{m}", tag="ss")
                        nc.sync.dma_start(
                            out=ss[:], in_=sg_d[b, m * 128 : (m + 1) * 128, nsl]
                        )
                        for k in range(K):
                            cs = cstg.tile([128, NHW], F32, name=f"cs_{b}_{nh}_{m}_{k}", tag="cs")
                            nc.sync.dma_start(
                                out=cs[:], in_=ch_d[k, m * 128 : (m + 1) * 128, nsl]
                            )
                            mt = mkp.tile([128, NHW], F32R, name=f"mk_{b}_{nh}_{m}_{k}", tag="mk")
                            nc.vector.tensor_mul(out=mt[:], in0=cs[:], in1=ss[:])
                            mks[(k, m)] = mt

                    # asm tiles for this (b, nh): one per n-chunk of 128
                    asms = [
                        asmp.tile([128, O * T], F32, name=f"asm_{b}_{nh}_{c4}", tag="asm")
                        for c4 in range(4)
                    ]

                    # ---- phase B ----
                    for t0 in range(TP):
                        ys = []
                        for k in range(K):
                            py = psy.tile([128, NHW], F32, name=f"py_{b}_{nh}_{t0}_{k}", tag="py")
                            for m in range(MT):
                                nc.tensor.matmul(
                                    py[:],
                                    xts[m][:]
                                    .rearrange("p (c t) -> p c t", t=T)[
                                        :, :, t0 : T : TP
                                    ],
                                    mks[(k, m)][:],
                                    start=(m == 0),
                                    stop=(m == MT - 1),
                                )
                            yk = yp.tile([128, NHW], BF16, name=f"yk_{b}_{nh}_{t0}_{k}", tag="yk")
                            nc.scalar.copy(out=yk[:], in_=py[:])
                            ys.append(yk)

                        for c4 in range(4):
                            po = pso.tile(
                                [128, 128], F32, name=f"po_{b}_{nh}_{t0}_{c4}", tag="po"
                            )
                            for k in range(K):
                                nc.tensor.matmul(
                                    po[:],
                                    ys[k][:, c4 * 128 : (c4 + 1) * 128],
                                    tb[:, k * 128 : (k + 1) * 128],
                                    start=(k == 0),
                                    stop=(k == K - 1),
                                )
                            nc.scalar.activation(
                                out=asms[c4][:].rearrange("p (o t) -> p o t", t=T)[
                                    :, :, t0 : T : TP
                                ],
                                in_=po[:],
                                func=mybir.ActivationFunctionType.Relu,
                            )

                    for c4 in range(4):
                        n0 = nh * NHW + c4 * 128
                        nc.sync.dma_start(
                            out=y_d[b, n0 : n0 + 128].rearrange("p o t -> p (o t)"),
                            in_=asms[c4][:],
                        )
    nc.compile()
    return nc


def _theta_blocks(theta: np.ndarray) -> np.ndarray:
    tb = np.zeros((K, 64, 2, 64, 2), dtype=np.float32)
    tb[:, :, 0, :, 0] = theta
    tb[:, :, 1, :, 1] = theta
    return tb.reshape(K, 128, 128).astype(ml_dtypes.bfloat16)


def kernel(x, STDG, cheb, theta):
    x = np.ascontiguousarray(np.asarray(x), dtype=np.float32)
    STDG = np.ascontiguousarray(np.asarray(STDG), dtype=np.float32)
    cheb = np.ascontiguousarray(np.asarray(cheb), dtype=np.float32)
    theta = np.asarray(theta, dtype=np.float32)

    if "nc" not in _cache:
        _cache["nc"] = build_nc()
    nc = _cache["nc"]

    tbk = _theta_blocks(theta)
    in_maps = [
        {
            "x": x[i * BL : (i + 1) * BL],
            "stdg": STDG[i * BL : (i + 1) * BL],
            "cheb": cheb,
            "thetab": tbk,
        }
        for i in range(NCORES)
    ]
    res = run_bass_kernel_spmd(nc, in_maps, list(range(NCORES)))
    out = np.concatenate([res.results[i]["y"] for i in range(NCORES)], axis=0)
    return out


# revision 3
# speedup vs baseline: 1.2021x; 1.2021x over previous
"""Chebyshev GCN kernel for Trainium2 (8 NeuronCores, data-parallel over batch).

Math (per batch b):
  masked[k] = cheb[k] * STDG[b]            (elementwise, [N,N])
  Y[k]      = masked[k].T @ x[b]           ([N, C*T] contraction over nodes m)
  out[b]    = relu( sum_k  Y[k] contracted with theta[k] over C )

Mapping per core (2 batches/core):
  stage 1 (PE, f32r): lhsT = x tiles [m_part=128, (c,t2)=128], rhs = masked
           tiles [m_part=128, n=512]  ->  psum Y [(c,t2)=128, n=512],
           accumulated over 8 m-tiles.  t2 = t-pair (t0, t0+12) so the
           stationary free dim packs c(64) x t(2) = 128 partitions out.
  stage 2 (PE, bf16): lhsT = Y[k] [(c,t2), n_chunk=128], rhs = block-diag
           theta [(c,t2)=128, (o,t2)=128] -> psum out [n=128, (o,t2)=128],
           accumulated over k.  ReLU fused into the PSUM->SBUF copy that
           scatters (o,t2) columns into a contiguous [n, (o,t)] tile.
"""

import numpy as np
import ml_dtypes

import concourse.bass as bass
import concourse.mybir as mybir
from concourse import bacc
from concourse.tile import TileContext
from concourse.bass_utils import run_bass_kernel_spmd

B, N, C, T, K, O = 16, 1024, 64, 24, 3, 64
NCORES = 8
BL = B // NCORES  # batches per core
MT = N // 128  # m tiles
TP = T // 2  # t-pairs
NH = 2  # n halves
NHW = N // NH  # 512
F32 = mybir.dt.float32
F32R = mybir.dt.float32r
BF16 = mybir.dt.bfloat16

_cache = {}


def build_nc():
    nc = bacc.Bacc("TRN2", target_bir_lowering=False, debug=False, num_devices=NCORES)
    x_d = nc.dram_tensor("x", [BL, N, C, T], F32, kind="ExternalInput")
    sg_d = nc.dram_tensor("stdg", [BL, N, N], F32, kind="ExternalInput")
    ch_d = nc.dram_tensor("cheb", [K, N, N], F32, kind="ExternalInput")
    tb_d = nc.dram_tensor("thetab", [K, 128, 128], BF16, kind="ExternalInput")
    y_d = nc.dram_tensor("y", [BL, N, C, T], F32, kind="ExternalOutput")

    with TileContext(nc) as tc:
        with (
            tc.tile_pool(name="xp", bufs=MT) as xp,
            tc.tile_pool(name="xstg", bufs=2) as xstg,
            tc.tile_pool(name="cstg", bufs=3) as cstg,
            tc.tile_pool(name="sstg", bufs=3) as sstg,
            tc.tile_pool(name="mk", bufs=K * MT + 9) as mkp,
            tc.tile_pool(name="yp", bufs=2 * K) as yp,
            tc.tile_pool(name="tbp", bufs=1) as tbp,
            tc.tile_pool(name="asmp", bufs=6) as asmp,
            tc.tile_pool(name="psy", bufs=2 * K, space="PSUM") as psy,
            tc.tile_pool(name="pso", bufs=2, space="PSUM") as pso,
        ):
            tb = tbp.tile([128, K * 128], BF16)
            for k in range(K):
                nc.sync.dma_start(out=tb[:, k * 128 : (k + 1) * 128], in_=tb_d[k])

            for b in range(BL):
                # ---- load x[b]: round to f32r and pack so each t-pair's
                # [128, (c,t2)=128] weight block is contiguous (fast LDW) ----
                xts = []
                for m in range(MT):
                    xs = xstg.tile([128, C * T], F32, name=f"xs_{b}_{m}", tag="xs")
                    nc.gpsimd.dma_start(
                        out=xs[:],
                        in_=x_d[b, m * 128 : (m + 1) * 128].rearrange(
                            "p c t -> p (c t)"
                        ),
                    )
                    xt = xp.tile([128, C * T], F32R, name=f"xt_{b}_{m}", tag="xt")
                    nc.vector.tensor_copy(
                        out=xt[:].rearrange("p (a c t2) -> p a c t2", a=TP, c=C),
                        in_=xs[:].rearrange("p (c t2 a) -> p a c t2", c=C, t2=2),
                    )
                    xts.append(xt)

                for nh in range(NH):
                    nsl = slice(nh * NHW, (nh + 1) * NHW)
                    # ---- phase A: masked tiles ----
                    mks = {}
                    for m in range(MT):
                        ss = sstg.tile([128, NHW], F32, name=f"ss_{b}_{nh}_{m}", tag="ss")
                        nc.sync.dma_start(
                            out=ss[:], in_=sg_d[b, m * 128 : (m + 1) * 128, nsl]
                        )
                        for k in range(K):
                            cs = cstg.tile([128, NHW], F32, name=f"cs_{b}_{nh}_{m}_{k}", tag="cs")
                            nc.sync.dma_start(
                                out=cs[:], in_=ch_d[k, m * 128 : (m + 1) * 128, nsl]
                            )
                            mt = mkp.tile([128, NHW], F32R, name=f"mk_{b}_{nh}_{m}_{k}", tag="mk")
                            nc.vector.tensor_mul(out=mt[:], in0=cs[:], in1=ss[:])
                            mks[(k, m)] = mt

                    # asm tiles for this (b, nh): one per n-chunk of 128
                    asms = [
                        asmp.tile([128, O * T], F32, name=f"asm_{b}_{nh}_{c4}", tag="asm")
                        for c4 in range(4)
                    ]

                    # ---- phase B ----
                    for t0 in range(TP):
                        ys = []
                        for k in range(K):
                            py = psy.tile([128, NHW], F32, name=f"py_{b}_{nh}_{t0}_{k}", tag="py")
                            for m in range(MT):
                                nc.tensor.matmul(
                                    py[:],
                                    xts[m][:, t0 * 128 : (t0 + 1) * 128],
                                    mks[(k, m)][:],
                                    start=(m == 0),
                                    stop=(m == MT - 1),
                                )
                            yk = yp.tile([128, NHW], BF16, name=f"yk_{b}_{nh}_{t0}_{k}", tag="yk")
                            nc.vector.tensor_copy(out=yk[:], in_=py[:])
                            ys.append(yk)

                        for c4 in range(4):
                            po = pso.tile(
                                [128, 128], F32, name=f"po_{b}_{nh}_{t0}_{c4}", tag="po"
                            )
                            for k in range(K):
                                nc.tensor.matmul(
                                    po[:],
                                    ys[k][:, c4 * 128 : (c4 + 1) * 128],
                                    tb[:, k * 128 : (k + 1) * 128],
                                    start=(k == 0),
                                    stop=(k == K - 1),
                                )
                            nc.scalar.activation(
                                out=asms[c4][:].rearrange("p (o t) -> p o t", t=T)[
                                    :, :, t0 : T : TP
                                ],
                                in_=po[:],
                                func=mybir.ActivationFunctionType.Relu,
                            )

                    for c4 in range(4):
                        n0 = nh * NHW + c4 * 128
                        nc.gpsimd.dma_start(
                            out=y_d[b, n0 : n0 + 128].rearrange("p o t -> p (o t)"),
                            in_=asms[c4][:],
                        )
    nc.compile()
    return nc


def _theta_blocks(theta: np.ndarray) -> np.ndarray:
    tb = np.zeros((K, 64, 2, 64, 2), dtype=np.float32)
    tb[:, :, 0, :, 0] = theta
    tb[:, :, 1, :, 1] = theta
    return tb.reshape(K, 128, 128).astype(ml_dtypes.bfloat16)


def kernel(x, STDG, cheb, theta):
    x = np.ascontiguousarray(np.asarray(x), dtype=np.float32)
    STDG = np.ascontiguousarray(np.asarray(STDG), dtype=np.float32)
    cheb = np.ascontiguousarray(np.asarray(cheb), dtype=np.float32)
    theta = np.asarray(theta, dtype=np.float32)

    if "nc" not in _cache:
        _cache["nc"] = build_nc()
    nc = _cache["nc"]

    tbk = _theta_blocks(theta)
    in_maps = [
        {
            "x": x[i * BL : (i + 1) * BL],
            "stdg": STDG[i * BL : (i + 1) * BL],
            "cheb": cheb,
            "thetab": tbk,
        }
        for i in range(NCORES)
    ]
    res = run_bass_kernel_spmd(nc, in_maps, list(range(NCORES)))
    out = np.concatenate([res.results[i]["y"] for i in range(NCORES)], axis=0)
    return out
